# revision 7
# baseline (speedup 1.0000x reference)
"""BiLSTM-CRF NLL kernel for 8 Trainium2 NeuronCores.

Strategy (v1): data-parallel over batch. Each of the 8 cores processes 4 of the
32 sequences end-to-end: embedding gather -> 2-layer BiLSTM -> tag projection
-> CRF (exp-domain forward scan). Device returns per-core emission-score sums
and log-partition values; host adds the input-only numerator terms and reduces
to the scalar token-mean NLL.

Everything LSTM runs in a fully transposed layout: gates G^T [4H, B] are
computed as 16x(128) M-chunks via PSUM-accumulated matmuls with the (reordered
i,f,o,g) weight tiles as the stationary operand and the fp16 state h^T as the
moving operand, so the per-step nonlinearities run on 128 partitions and the
next step's h^T needs no transpose.
"""
import sys
from contextlib import ExitStack

sys.path.insert(0, "/opt/trn_rl_repo")

import numpy as np
import concourse.bass as bass
import concourse.tile as tile
import concourse.mybir as mybir
from concourse.bass_utils import run_bass_kernel_spmd

AF = mybir.ActivationFunctionType
ALU = mybir.AluOpType
FP16 = mybir.dt.float16
FP32 = mybir.dt.float32
I32 = mybir.dt.int32

B, S, E, H, L, T, V = 32, 512, 300, 512, 2, 9, 30000
N_CORES = 8
BL = B // N_CORES        # sequences per core
NTOK = BL * S            # tokens per core
EP = 384                 # embedding dim padded to 3*128
NC_E = EP // 128         # emb K-chunks
NC_G = 16                # gate M-chunks (4H/128)
NC_H = 4                 # hidden K-chunks (H/128)
W = 16                   # recurrence window (steps per For_i iteration)
NW = S // W
RESC = 8                 # CRF rescale interval


def split_drain_waits(nc, cap=1):
    """This walrus build allows only one sync-wait on CTRL-class instructions
    (Drain/NoOp). Tile's epilogue piles one wait per clock domain onto them,
    so move the extras onto single-wait NoOps placed just before (same engine,
    program order preserves semantics)."""
    n_new = 0
    for fn in nc.m.functions:
        for bb in fn.blocks:
            out = []
            for inst in bb.instructions:
                si = inst.sync_info
                nw = len(si.on_wait) if si and si.on_wait else 0
                if nw > cap:
                    waits = list(si.on_wait)
                    for w_ in waits[:-cap]:
                        nop = mybir.InstNoOp(
                            name=f"waitsplit_{n_new}_{inst.name}",
                            engine=inst.engine,
                            sync_info=mybir.SyncInfo(on_wait=[w_], on_update=[]),
                            bass_nofuse=True,
                        )
                        n_new += 1
                        out.append(nop)
                    inst.sync_info = mybir.SyncInfo(
                        on_wait=waits[-cap:], on_update=list(si.on_update or [])
                    )
                out.append(inst)
            bb.instructions[:] = out
    return n_new


def build_nc(debug=False, repeat=1):
    nc = bass.Bass("TRN2", target_bir_lowering=False, debug=False,
                   num_devices=N_CORES)

    # ---- I/O ----
    embT_in = nc.declare_dram_parameter("embT_pre", [NC_E, 128, NTOK], FP16, isOutput=False)
    wih0_in = nc.declare_dram_parameter("wihT0", [2, NC_E, 128, 2048], FP16, isOutput=False)
    whh0_in = nc.declare_dram_parameter("whhT0", [2, NC_H, 128, 2048], FP16, isOutput=False)
    b0_in = nc.declare_dram_parameter("bias0", [2, NC_G, 128, 1], FP32, isOutput=False)
    wih1_in = nc.declare_dram_parameter("wihT1", [2, 2 * NC_H, 128, 2048], FP16, isOutput=False)
    whh1_in = nc.declare_dram_parameter("whhT1", [2, NC_H, 128, 2048], FP16, isOutput=False)
    b1_in = nc.declare_dram_parameter("bias1", [2, NC_G, 128, 1], FP32, isOutput=False)
    wtag_in = nc.declare_dram_parameter("wtagT", [2 * NC_H, 128, T], FP16, isOutput=False)
    btag_in = nc.declare_dram_parameter("btag", [T, 1], FP32, isOutput=False)
    eexp_in = nc.declare_dram_parameter("E_exp", [T, T], FP32, isOutput=False)
    est_in = nc.declare_dram_parameter("exp_start", [T, 1], FP32, isOutput=False)
    een_in = nc.declare_dram_parameter("exp_end", [T, 1], FP32, isOutput=False)
    oneh_in = nc.declare_dram_parameter("onehot", [T, NTOK], FP32, isOutput=False)
    ones9_in = nc.declare_dram_parameter("ones9", [T, 1], FP32, isOutput=False)
    ones19_in = nc.declare_dram_parameter("ones19", [1, T], FP32, isOutput=False)

    crf_out = nc.declare_dram_parameter("crf", [4, BL], FP32, isOutput=True)
    em_out = nc.declare_dram_parameter("em_dbg", [T, NTOK], FP32, isOutput=True)

    # ---- DRAM scratch ----
    # X[d][c][b][p][s] : projected inputs (x@Wih.T + biases), transposed layout
    X0 = nc.dram_tensor("X0", [2, NC_G, BL, 128, S], FP32)
    X1 = nc.dram_tensor("X1", [2, NC_G, BL, 128, S], FP32)
    # h[d][c][b][p][s] : hidden states, transposed layout
    h0 = nc.dram_tensor("h0s", [2, NC_H, BL, 128, S], FP16,
                        kind="ExternalOutput" if debug else "Internal")
    h1 = nc.dram_tensor("h1s", [2, NC_H, BL, 128, S], FP16,
                        kind="ExternalOutput" if debug else "Internal")

    with tile.TileContext(nc) as tc:
        ctx_pools = []

        # ============ persistent constants ============
        with tc.tile_pool(name="const", bufs=1) as cpool:
            b0_sb = cpool.tile([128, 2 * NC_G], FP32)
            nc.sync.dma_start(
                b0_sb[:].rearrange("p (d c) -> p d c", d=2),
                b0_in[:].rearrange("d c p one -> p d (c one)"),
            )
            b1_sb = cpool.tile([128, 2 * NC_G], FP32)
            nc.sync.dma_start(
                b1_sb[:].rearrange("p (d c) -> p d c", d=2),
                b1_in[:].rearrange("d c p one -> p d (c one)"),
            )
            btag_sb = cpool.tile([T, 1], FP32)
            nc.sync.dma_start(btag_sb[:], btag_in[:])

            _reps = ExitStack()
            if repeat > 1:
                _reps.enter_context(tc.For_i(0, repeat, 1))

            # ============ phase 0: load pre-transposed embeddings ============
            with (
                tc.tile_pool(name="embT", bufs=1) as epool,
            ):
                embT = epool.tile([128, NC_E * NTOK], FP16)  # [p, k, tok]
                nc.sync.dma_start(
                    embT[:].rearrange("p (k t) -> p k t", k=NC_E),
                    embT_in[:].rearrange("k p t -> p k t"),
                )

                # ============ phase 1: layer-0 input projection ============
                with (
                    tc.tile_pool(name="w0", bufs=1) as w0pool,
                    tc.tile_pool(name="xev", bufs=3) as xev,
                    tc.tile_pool(name="pps", bufs=2, space="PSUM") as pps,
                ):
                    wih0 = w0pool.tile([128, 2 * NC_E * 2048], FP16)  # [p,(d k) m]
                    nc.sync.dma_start(
                        wih0[:].rearrange("p (d k m) -> p d k m", d=2, k=NC_E),
                        wih0_in[:].rearrange("d k p m -> p d k m"),
                    )
                    for d in range(2):
                        for b in range(BL):
                            for c in range(NC_G):
                                ps = pps.tile([128, S], FP32, tag="pp")
                                for k in range(NC_E):
                                    nc.tensor.matmul(
                                        ps[:],
                                        lhsT=wih0[:, (d * NC_E + k) * 2048 + 128 * c:
                                                  (d * NC_E + k) * 2048 + 128 * (c + 1)],
                                        rhs=embT[:, k * NTOK + b * S: k * NTOK + (b + 1) * S],
                                        start=(k == 0), stop=(k == NC_E - 1),
                                    )
                                xe = xev.tile([128, S], FP32, tag="xe")
                                nc.scalar.activation(xe[:], ps[:], AF.Identity,
                                                     bias=b0_sb[:, d * NC_G + c: d * NC_G + c + 1])
                                nc.sync.dma_start(X0[d, c, b], xe[:])

            # ============ phase 2: layer-0 recurrence ============
            _recurrence(nc, tc, X0, h0, whh0_in)

            # ============ phase 3: layer-1 input projection ============
            with (
                tc.tile_pool(name="w1", bufs=1) as w1pool,
                tc.tile_pool(name="hrhs", bufs=3) as hrhs,
                tc.tile_pool(name="xev1", bufs=3) as xev1,
                tc.tile_pool(name="pps1", bufs=2, space="PSUM") as pps1,
            ):
                wih1 = w1pool.tile([128, 2 * 2 * NC_H * 2048], FP16)
                nc.sync.dma_start(
                    wih1[:].rearrange("p (d k m) -> p d k m", d=2, k=2 * NC_H),
                    wih1_in[:].rearrange("d k p m -> p d k m"),
                )
                for b in range(BL):
                    hts = []
                    for k in range(2 * NC_H):
                        ht = hrhs.tile([128, S], FP16, tag=f"hr{k % 3}")
                        nc.sync.dma_start(ht[:], h0[k // NC_H, k % NC_H, b])
                        hts.append(ht)
                    for d in range(2):
                        for c in range(NC_G):
                            ps = pps1.tile([128, S], FP32, tag="pp1")
                            for k in range(2 * NC_H):
                                nc.tensor.matmul(
                                    ps[:],
                                    lhsT=wih1[:, (d * 2 * NC_H + k) * 2048 + 128 * c:
                                              (d * 2 * NC_H + k) * 2048 + 128 * (c + 1)],
                                    rhs=hts[k][:],
                                    start=(k == 0), stop=(k == 2 * NC_H - 1),
                                )
                            xe = xev1.tile([128, S], FP32, tag="xe1")
                            nc.scalar.activation(xe[:], ps[:], AF.Identity,
                                                 bias=b1_sb[:, d * NC_G + c: d * NC_G + c + 1])
                            nc.sync.dma_start(X1[d, c, b], xe[:])

            # ============ phase 4: layer-1 recurrence ============
            _recurrence(nc, tc, X1, h1, whh1_in)

            # ============ phase 5: tag projection + CRF ============
            with (
                tc.tile_pool(name="tag", bufs=1) as tgpool,
                tc.tile_pool(name="hr2", bufs=3) as hr2,
                tc.tile_pool(name="crf", bufs=1) as crf,
                tc.tile_pool(name="crfps", bufs=1, space="PSUM") as crfps,
            ):
                wtag = tgpool.tile([128, 2 * NC_H * T], FP16)
                nc.sync.dma_start(
                    wtag[:].rearrange("p (k t) -> p k t", k=2 * NC_H),
                    wtag_in[:].rearrange("k p t -> p k t"),
                )
                emT = crf.tile([T, NTOK], FP32)
                for b in range(BL):
                    ps = crfps.tile([T, S], FP32, tag="emps")
                    for k in range(2 * NC_H):
                        ht = hr2.tile([128, S], FP16, tag=f"h2{k % 3}")
                        nc.sync.dma_start(ht[:], h1[k // NC_H, k % NC_H, b])
                        nc.tensor.matmul(
                            ps[:], lhsT=wtag[:, k * T:(k + 1) * T], rhs=ht[:],
                            start=(k == 0), stop=(k == 2 * NC_H - 1),
                        )
                    nc.scalar.activation(emT[:, b * S:(b + 1) * S], ps[:],
                                         AF.Identity, bias=btag_sb[:, 0:1])
                nc.sync.dma_start(em_out[:], emT[:])

                # ---- CRF pieces ----
                eexp = crf.tile([T, T], FP32)
                nc.sync.dma_start(eexp[:], eexp_in[:])
                est = crf.tile([T, 1], FP32)
                nc.sync.dma_start(est[:], est_in[:])
                een = crf.tile([T, 1], FP32)
                nc.sync.dma_start(een[:], een_in[:])
                oneh = crf.tile([T, NTOK], FP32)
                nc.sync.dma_start(oneh[:], oneh_in[:])
                ones9 = crf.tile([T, 1], FP32)
                nc.sync.dma_start(ones9[:], ones9_in[:])
                ones19 = crf.tile([1, T], FP32)
                nc.sync.dma_start(ones19[:], ones19_in[:])

                # numerator emission sum: sum_t em[b, t, tag[b,t]]
                prod = crf.tile([T, NTOK], FP32)
                nc.vector.tensor_mul(prod[:], emT[:], oneh[:])
                esum9 = crf.tile([T, BL], FP32)
                for b in range(BL):
                    nc.vector.tensor_reduce(
                        esum9[:, b:b + 1], prod[:, b * S:(b + 1) * S],
                        mybir.AxisListType.X, ALU.add)
                esump = crfps.tile([1, BL], FP32, tag="esum")
                nc.tensor.matmul(esump[:], lhsT=ones9[:], rhs=esum9[:],
                                 start=True, stop=True)
                esum_sb = crf.tile([1, BL], FP32)
                nc.scalar.activation(esum_sb[:], esump[:], AF.Identity)
                nc.sync.dma_start(crf_out[0:1, :], esum_sb[:])

                # exp(emissions)
                expe = crf.tile([T, NTOK], FP32)
                nc.scalar.activation(expe[:], emT[:], AF.Exp)

                # forward scan in the exp domain with periodic rescaling
                s_sb = crf.tile([T, BL], FP32)
                lacc = crf.tile([1, BL], FP32)
                nc.vector.memset(lacc[:], 0.0)
                # s0 = exp(start) * expe[:, t=0]
                nc.vector.tensor_mul(
                    s_sb[:], expe[:, 0:NTOK:S], est[:].to_broadcast([T, BL]))
                tmp9 = crf.tile([T, BL], FP32)
                ln4 = crf.tile([1, BL], FP32)
                for t in range(1, S):
                    sp = crfps.tile([T, BL], FP32, tag="scan")
                    nc.tensor.matmul(sp[:], lhsT=eexp[:], rhs=s_sb[:],
                                     start=True, stop=True)
                    nc.vector.tensor_mul(
                        s_sb[:], sp[:],
                        expe[:].rearrange("q (b s) -> q b s", b=BL)[:, :, t])
                    if t % RESC == 0:
                        mm = crfps.tile([1, BL], FP32, tag="resc")
                        nc.tensor.matmul(mm[:], lhsT=ones9[:], rhs=s_sb[:],
                                         start=True, stop=True)
                        minv = crf.tile([1, BL], FP32, tag="minv")
                        nc.vector.reciprocal(minv[:], mm[:])
                        nc.scalar.activation(ln4[:], mm[:], AF.Ln)
                        nc.vector.tensor_add(lacc[:], lacc[:], ln4[:])
                        bc = crfps.tile([T, BL], FP32, tag="bc")
                        nc.tensor.matmul(bc[:], lhsT=ones19[:], rhs=minv[:],
                                         start=True, stop=True)
                        nc.vector.tensor_mul(s_sb[:], s_sb[:], bc[:])
                # denom = lacc + ln(sum_j s_j * exp(end_j))
                nc.vector.tensor_mul(tmp9[:], s_sb[:], een[:].to_broadcast([T, BL]))
                mm = crfps.tile([1, BL], FP32, tag="fin")
                nc.tensor.matmul(mm[:], lhsT=ones9[:], rhs=tmp9[:],
                                 start=True, stop=True)
                lnf = crf.tile([1, BL], FP32)
                nc.scalar.activation(lnf[:], mm[:], AF.Ln)
                den = crf.tile([1, BL], FP32)
                nc.vector.tensor_add(den[:], lnf[:], lacc[:])
                nc.sync.dma_start(crf_out[1:2, :], den[:])
                nc.sync.dma_start(crf_out[2:3, :], lacc[:])
                nc.sync.dma_start(crf_out[3:4, :], lnf[:])
            _reps.close()

    split_drain_waits(nc)
    return nc


def _recurrence(nc, tc, X, hout, whh_in):
    """Both directions of one BiLSTM layer, interleaved step-by-step.
    X/hout: DRAM [2, NC, BL, 128, S]; whh_in: DRAM [2, NC_H, 128, 2048]."""
    CB = NC_G * BL            # gate chunk-cols in PSUM
    with (
        tc.tile_pool(name="whh", bufs=1) as wpool,
        tc.tile_pool(name="st", bufs=1) as st,
        tc.tile_pool(name="xw", bufs=2) as xw,
        tc.tile_pool(name="hw", bufs=2) as hw,
        tc.tile_pool(name="el", bufs=3) as el,
        tc.tile_pool(name="gps", bufs=2, space="PSUM") as gps,
    ):
        whh = wpool.tile([128, 2 * NC_H * 2048], FP16)
        nc.sync.dma_start(
            whh[:].rearrange("p (d k m) -> p d k m", d=2, k=NC_H),
            whh_in[:].rearrange("d k p m -> p d k m"),
        )
        hT = [st.tile([128, NC_H * BL], FP16, tag=f"hT{d}", name=f"hT{d}")
              for d in range(2)]
        cst = [st.tile([128, NC_H * BL], FP32, tag=f"c{d}", name=f"c{d}")
               for d in range(2)]
        for d in range(2):
            nc.vector.memset(hT[d][:], 0.0)
            nc.vector.memset(cst[d][:], 0.0)

        def step(d, xwin, hwin, j):
            G = gps.tile([128, CB], FP32, tag=f"G{d}")
            for c in range(NC_G):
                for k in range(NC_H):
                    nc.tensor.matmul(
                        G[:, BL * c:BL * (c + 1)],
                        lhsT=whh[:, (d * NC_H + k) * 2048 + 128 * c:
                                 (d * NC_H + k) * 2048 + 128 * (c + 1)],
                        rhs=hT[d][:, BL * k:BL * (k + 1)],
                        start=(k == 0), stop=(k == NC_H - 1),
                    )
            t1 = el.tile([128, CB], FP32, tag=f"t1{d}")
            nc.vector.tensor_add(
                t1[:], G[:],
                xwin[:].rearrange("p (cb w) -> p cb w", w=W)[:, :, j])
            s1 = el.tile([128, 3 * NC_H * BL], FP32, tag=f"s1{d}")
            nc.scalar.activation(s1[:], t1[:, 0:3 * NC_H * BL], AF.Sigmoid)
            s2 = el.tile([128, NC_H * BL], FP32, tag=f"s2{d}")
            nc.scalar.activation(s2[:], t1[:, 3 * NC_H * BL:CB], AF.Tanh)
            nbl = NC_H * BL
            nc.vector.tensor_mul(cst[d][:], cst[d][:], s1[:, nbl:2 * nbl])
            tm = el.tile([128, nbl], FP32, tag=f"tm{d}")
            nc.vector.tensor_mul(tm[:], s1[:, 0:nbl], s2[:])
            nc.vector.tensor_add(cst[d][:], cst[d][:], tm[:])
            s3 = el.tile([128, nbl], FP32, tag=f"s3{d}")
            nc.scalar.activation(s3[:], cst[d][:], AF.Tanh)
            nc.vector.tensor_mul(hT[d][:], s1[:, 2 * nbl:3 * nbl], s3[:])
            # stash into the store window at ascending absolute position
            pos = j if d == 0 else W - 1 - j
            nc.vector.tensor_copy(
                hwin[:].rearrange("p (cb w) -> p cb w", w=W)[:, :, pos], hT[d][:])

        with tc.For_i(0, NW, 1) as i:
            xws, hws = [], []
            for d in range(2):
                xwin = xw.tile([128, CB * W], FP32, tag=f"xw{d}")
                base = i * W if d == 0 else (S - W) - i * W
                nc.sync.dma_start(
                    xwin[:].rearrange("p (c b w) -> p c b w", c=NC_G, b=BL),
                    X[d].rearrange("c b p s -> p c b s")[:, :, :, bass.ds(base, W)],
                )
                hws.append(hw.tile([128, NC_H * BL * W], FP16, tag=f"hw{d}", name=f"hwin{d}"))
                xws.append(xwin)
            for j in range(W):
                step(0, xws[0], hws[0], j)
                step(1, xws[1], hws[1], j)
            for d in range(2):
                base = i * W if d == 0 else (S - W) - i * W
                nc.sync.dma_start(
                    hout[d].rearrange("c b p s -> p c b s")[:, :, :, bass.ds(base, W)],
                    hws[d][:].rearrange("p (c b w) -> p c b w", c=NC_H, b=BL),
                )


# ================= host side =================

def _prep_inputs(ids, target_tag, embed_table, lstm_params, W_tag, b_tag,
                 start_trans, end_trans, trans):
    """Build per-core input maps (numpy only)."""
    ids = np.asarray(ids).astype(np.int32)
    tgt = np.asarray(target_tag).astype(np.int64)
    perm = np.concatenate([np.arange(0, 2 * H), np.arange(3 * H, 4 * H),
                           np.arange(2 * H, 3 * H)])  # i,f,g,o -> i,f,o,g

    def prep_layer(layer, in_dim, in_pad):
        wihT = np.zeros((2, in_pad // 128, 128, 2048), np.float16)
        whhT = np.zeros((2, NC_H, 128, 2048), np.float16)
        bias = np.zeros((2, NC_G, 128, 1), np.float32)
        for d in range(2):
            Wih, Whh, bih, bhh = lstm_params[layer][d]
            Wih = np.asarray(Wih)[perm]          # [2048, in_dim]
            Whh = np.asarray(Whh)[perm]          # [2048, 512]
            bsum = (np.asarray(bih) + np.asarray(bhh))[perm]
            wt = np.zeros((in_pad, 2048), np.float32)
            wt[:in_dim] = Wih.T
            wihT[d] = wt.reshape(in_pad // 128, 128, 2048).astype(np.float16)
            whhT[d] = Whh.T.reshape(NC_H, 128, 2048).astype(np.float16)
            bias[d] = bsum.reshape(NC_G, 128, 1).astype(np.float32)
        return wihT, whhT, bias

    wih0, whh0, b0 = prep_layer(0, E, EP)
    wih1, whh1, b1 = prep_layer(1, 2 * H, 2 * H)

    wtagT = np.asarray(W_tag).T.reshape(2 * NC_H, 128, T).astype(np.float16)
    btag = np.asarray(b_tag).reshape(T, 1).astype(np.float32)
    eexp = np.exp(np.asarray(trans)).astype(np.float32)
    est = np.exp(np.asarray(start_trans)).reshape(T, 1).astype(np.float32)
    een = np.exp(np.asarray(end_trans)).reshape(T, 1).astype(np.float32)
    ones9 = np.ones((T, 1), np.float32)
    ones19 = np.ones((1, T), np.float32)
    emb = np.asarray(embed_table).astype(np.float32)

    in_maps = []
    for core in range(N_CORES):
        bsl = slice(core * BL, (core + 1) * BL)
        ids_c = ids[bsl].reshape(-1)                       # [NTOK]
        rows = emb[ids_c]                                  # [NTOK, E]
        embT_pre = np.zeros((NC_E, 128, NTOK), np.float16)
        embT_pre.reshape(EP, NTOK)[:E] = rows.T.astype(np.float16)
        tgt_c = tgt[bsl].reshape(-1)
        oneh = np.zeros((T, NTOK), np.float32)
        oneh[tgt_c, np.arange(NTOK)] = 1.0
        in_maps.append({
            "embT_pre": embT_pre,
            "wihT0": wih0, "whhT0": whh0, "bias0": b0,
            "wihT1": wih1, "whhT1": whh1, "bias1": b1,
            "wtagT": wtagT, "btag": btag,
            "E_exp": eexp, "exp_start": est, "exp_end": een,
            "onehot": oneh, "ones9": ones9, "ones19": ones19,
        })
    return in_maps


_NC_CACHE = {}


def get_nc(debug=False):
    if debug not in _NC_CACHE:
        _NC_CACHE[debug] = build_nc(debug)
    return _NC_CACHE[debug]


def kernel(ids, mask, token_type_ids, target_tag, embed_table, lstm_params,
           W_tag, b_tag, start_trans, end_trans, trans, _debug=False,
           _results_hook=None):
    in_maps = _prep_inputs(ids, target_tag, embed_table, lstm_params,
                           W_tag, b_tag, start_trans, end_trans, trans)
    nc = get_nc(_debug)
    res = run_bass_kernel_spmd(nc, in_maps, list(range(N_CORES)))
    if _results_hook is not None:
        _results_hook(res)

    # host: numerator terms that depend only on inputs
    tgt = np.asarray(target_tag).astype(np.int64)
    maskf = np.asarray(mask).astype(np.float32)
    trans = np.asarray(trans)
    num = np.asarray(start_trans)[tgt[:, 0]].astype(np.float64)
    num += (maskf[:, 1:] * trans[tgt[:, :-1], tgt[:, 1:]]).sum(axis=1)
    seq_ends = maskf.astype(bool).sum(axis=1).astype(np.int64) - 1
    last = np.take_along_axis(tgt, seq_ends[:, None], axis=1)[:, 0]
    num += np.asarray(end_trans)[last]

    llh_sum = 0.0
    for core in range(N_CORES):
        crf = res.results[core]["crf"]
        em_sum, denom = crf[0].astype(np.float64), crf[1].astype(np.float64)
        bsl = slice(core * BL, (core + 1) * BL)
        llh_sum += (num[bsl] + em_sum - denom).sum()
    loss = -(llh_sum / maskf.sum())
    return np.float32(loss)
